# revision 17
# baseline (speedup 1.0000x reference)
"""Trainium2 Bass kernel for nn_DecoderRNN (attention LSTM decoder + vocab projection).

Strategy (8 NeuronCores):
  - The 63-step LSTM/attention recurrence is replicated on all cores (identical
    SPMD program); the dominant output projection (T*B, H) x (H, V) is sharded
    over the vocab dimension (V/8 = 1250 logit columns per core). No collectives.
  - fp8-e4m3 + DoubleRow perf mode (2 contraction rows per partition, halving
    the instruction stream) for every recurrence GEMM: gates (x@Cx,
    attended@Ca, h@W_hh.T), attention scores, and the output projection.
    Fold matrices Cx/Ca and the step-0 gates are precomputed on the host in
    fp32.
  - Gate columns are ordered [g|i|f|o] and each 512-wide gate lives in its OWN
    single-bank PSUM tile, so tanh(g)/sigmoid(i) start as soon as their chunk
    of the gate GEMM finishes instead of after the full stream.
  - Per-step x-contributions (PA, PX) and all biases are accumulated directly
    into those PSUM banks one step ahead (start/stop accumulation groups).
  - sigmoid(x) = 0.5*tanh(x/2)+0.5 keeps every activation on the exp/tanh
    table: zero ACT table reloads in the loop.
  - h is packed column-wise (feature-major) into staging tiles; the output
    projection runs on 128-row batches, spread across steps' PE idle windows;
    its PSUM->SBUF copies run on the ACT engine (Copy needs no table).
  - Logits are written bf16, valid rows only; the host zero-fills, upcasts,
    and adds the output bias.  Ragged lengths are baked into the instruction
    stream.
"""

import os
import sys

import numpy as np

for _p in ("/opt/trn_rl_repo", "/root/.axon_site/_ro/trn_rl_repo"):
    if os.path.isdir(_p) and _p not in sys.path:
        sys.path.insert(0, _p)

import ml_dtypes
import concourse.bass as bass
import concourse.tile as tile
from concourse import bacc, mybir
from concourse.bass_utils import run_bass_kernel_spmd
from concourse.masks import make_identity

F32 = mybir.dt.float32
BF16 = mybir.dt.bfloat16
F8 = mybir.dt.float8e4
I32 = mybir.dt.int32
ADD = mybir.AluOpType.add
MULT = mybir.AluOpType.mult
TANH = mybir.ActivationFunctionType.Tanh
EXP = mybir.ActivationFunctionType.Exp
COPY = mybir.ActivationFunctionType.Copy
DR = mybir.MatmulPerfMode.DoubleRow
NP_BF16 = ml_dtypes.bfloat16
NP_F8 = np.dtype(mybir.dt.np(F8))

B, T, E, H, A, V = 128, 64, 512, 512, 512, 10000
G4 = 4 * H                      # 2048
NCORES = 8
VS = V // NCORES                # 1250 vocab columns per core
P = 128

KE = E // P                     # 4 k-tiles over E
KH = H // P
KA = A // P
MA = A // P                     # A m-tiles (feature-major attention)
NCH = 4                         # four 512-wide gate chunks: [g|i|f|o]


def _flush_plan(n_t):
    """Pack per-step h rows into 128-row batches for the output projection."""
    plan = []          # per t: (col0, flush_before: segments or None)
    segs = []
    pos = 0
    for t in range(T):
        nt = int(n_t[t])
        flush = None
        if pos + nt > P:
            flush = segs
            segs = []
            pos = 0
        plan.append((pos, flush))
        segs.append((t, pos, pos + nt))
        pos += nt
    return plan, segs  # segs = final leftover batch


def _build_nc(n_t):
    nc = bacc.Bacc("TRN2", target_bir_lowering=False, debug=False,
                   num_devices=NCORES)

    # ---------------- I/O ----------------
    cnn_T = nc.declare_dram_parameter("cnn_T", [A, B], BF16, isOutput=False)
    caps = nc.declare_dram_parameter("caps", [T, B], I32, isOutput=False)
    emb_W = nc.declare_dram_parameter("emb_W", [V, E], BF16, isOutput=False)
    awh_d = nc.declare_dram_parameter("awh", [H, A], F8, isOutput=False)
    awx_d = nc.declare_dram_parameter("awx", [E, A], F8, isOutput=False)
    attb_row = nc.declare_dram_parameter("attb_row", [1, A], BF16, isOutput=False)
    cx8_d = nc.declare_dram_parameter("cx8", [E, G4], F8, isOutput=False)
    ca8_d = nc.declare_dram_parameter("ca8", [A, G4], F8, isOutput=False)
    whh8_d = nc.declare_dram_parameter("whh8", [H, G4], F8, isOutput=False)
    bc_row = nc.declare_dram_parameter("bc_row", [1, G4], BF16, isOutput=False)
    g0_d = nc.declare_dram_parameter("g0", [B, G4], F32, isOutput=False)
    owt_d = nc.declare_dram_parameter("owt", [H, VS], BF16, isOutput=False)
    out = nc.declare_dram_parameter("out", [T, B, VS], BF16, isOutput=True)

    plan, final_segs = _flush_plan(n_t)

    with tile.TileContext(nc) as tc:
        with (
            tc.tile_pool(name="consts", bufs=1) as consts,
            tc.tile_pool(name="state", bufs=1) as state,
            tc.tile_pool(name="work", bufs=2) as work,
            tc.tile_pool(name="xstream", bufs=3) as xstream,
            tc.tile_pool(name="ps_g", bufs=1, space="PSUM") as ps_g,    # 4 banks
            tc.tile_pool(name="ps_s", bufs=2, space="PSUM") as ps_s,    # 2 banks
            tc.tile_pool(name="ps_tr", bufs=1, space="PSUM") as ps_tr,  # 1 bank
            tc.tile_pool(name="ps_o", bufs=1, space="PSUM") as ps_o,    # 1 bank
        ):
            # ---------------- weight / const loads (two HWDGE queues) ----------------
            ident16 = consts.tile([P, P], BF16)
            make_identity(nc, ident16)
            ones_bf = consts.tile([P, 1], BF16)
            nc.vector.memset(ones_bf, 1.0)

            def load3(dst, dram_ap):
                nc.sync.dma_start(dst, dram_ap.rearrange("(k p) n -> p k n", p=P))

            def load3b(dst, dram_ap):
                nc.scalar.dma_start(dst, dram_ap.rearrange("(k p) n -> p k n", p=P))

            g0_sb = consts.tile([P, G4], F32)
            nc.sync.dma_start(g0_sb, g0_d[:, :])
            toks = state.tile([B, T], I32)
            nc.sync.dma_start(toks, caps[:, :].rearrange("t b -> b t"))
            cnn_sb = consts.tile([P, KA, B], BF16)
            load3(cnn_sb, cnn_T[:, :])
            attb_sb = consts.tile([1, A], BF16)
            nc.sync.dma_start(attb_sb, attb_row[:, :])
            bc_sb = consts.tile([1, G4], BF16)
            nc.sync.dma_start(bc_sb, bc_row[:, :])

            awh_sb = state.tile([P, KH, A], F8)
            load3(awh_sb, awh_d[:, :])
            awx_sb = state.tile([P, KE, A], F8)
            load3(awx_sb, awx_d[:, :])
            cx8_sb = state.tile([P, KE, G4], F8)
            load3(cx8_sb, cx8_d[:, :])
            ca8_sb = state.tile([P, KA, G4], F8)
            load3b(ca8_sb, ca8_d[:, :])
            whh8_sb = state.tile([P, KH, G4], F8)
            load3b(whh8_sb, whh8_d[:, :])
            owt_sb = state.tile([P, KH, VS], BF16)
            load3b(owt_sb, owt_d[:, :])

            # recurrent state
            c_sb = state.tile([P, H], BF16)           # c, B-major
            stages = [state.tile([P, KH, P], BF16, name=f"stage{i}")
                      for i in range(2)]
            stages8 = [state.tile([P, KH, P], F8, name=f"stage8_{i}")
                       for i in range(2)]

            ones_row = ones_bf[0:1, 0:1]

            # ---------------- helpers ----------------
            def fetch_x(t):
                """Gather x_t embeddings; bf16 [E(part), KE, B] + fp8 cast."""
                xg = xstream.tile([P, E], BF16, tag="xg")
                nc.gpsimd.indirect_dma_start(
                    out=xg, out_offset=None, in_=emb_W[:, :],
                    in_offset=bass.IndirectOffsetOnAxis(ap=toks[:, t - 1:t], axis=0))
                xT = xstream.tile([P, KE, B], BF16, tag="xT")
                nc.sync.dma_start_transpose(xT, xg)
                x8 = xstream.tile([P, KE, B], F8, tag="x8")
                nc.vector.tensor_copy(x8, xT)
                return xT, x8

            def start_scores(t, x8):
                """New PSUM score tile for step t: att_b + PA (fp8 DoubleRow)."""
                nt = int(n_t[t])
                S = ps_s.tile([P, MA, B], F32, tag="att")
                for m in range(MA):
                    nc.tensor.matmul(S[:, m, 0:nt],
                                     attb_sb[0:1, m * P:(m + 1) * P],
                                     ones_row.to_broadcast([1, nt]),
                                     start=True, stop=False)
                    for j in range(KE // 2):
                        nc.tensor.matmul(S[:, m, 0:nt],
                                         awx_sb[:, 2 * j:2 * j + 2, m * P:(m + 1) * P],
                                         x8[:, 2 * j:2 * j + 2, 0:nt],
                                         start=False, stop=False, perf_mode=DR)
                return S

            def start_gates(t, x8):
                """Two new 2-bank PSUM gate tiles ([g|i] and [f|o]) for step t,
                seeded with bc + PX (fp8 DoubleRow)."""
                nt = int(n_t[t])
                Gs = []
                for half in range(2):
                    Gh = ps_g.tile([P, 1024], F32, tag=f"g{half}", name=f"g{half}")
                    for ci in range(2):
                        ns = slice((2 * half + ci) * 512, (2 * half + ci + 1) * 512)
                        rg = slice(ci * 512, (ci + 1) * 512)
                        nc.tensor.matmul(Gh[0:nt, rg], ones_row.to_broadcast([1, nt]),
                                         bc_sb[0:1, ns], start=True, stop=False)
                        for j in range(KE // 2):
                            nc.tensor.matmul(Gh[0:nt, rg], x8[:, 2 * j:2 * j + 2, 0:nt],
                                             cx8_sb[:, 2 * j:2 * j + 2, ns],
                                             start=False, stop=False, perf_mode=DR)
                    Gs.append(Gh)
                return Gs

            def gates_finish(t, Gs, attn8, hstage8, hcol):
                """+= attended @ Ca + h @ W_hh.T, one gate chunk at a time so
                downstream ACTs start as early as possible."""
                nt = int(n_t[t])
                for ci in range(NCH):
                    ns = slice(ci * 512, (ci + 1) * 512)
                    Gc = Gs[ci // 2]
                    rg = slice((ci % 2) * 512, (ci % 2 + 1) * 512)
                    for j in range(KA // 2):
                        nc.tensor.matmul(Gc[0:nt, rg], attn8[:, 2 * j:2 * j + 2, 0:nt],
                                         ca8_sb[:, 2 * j:2 * j + 2, ns],
                                         start=False, stop=False, perf_mode=DR)
                    for j in range(KH // 2):
                        nc.tensor.matmul(Gc[0:nt, rg],
                                         hstage8[:, 2 * j:2 * j + 2, hcol:hcol + nt],
                                         whh8_sb[:, 2 * j:2 * j + 2, ns],
                                         start=False, stop=(j == KH // 2 - 1),
                                         perf_mode=DR)

            def pointwise_compute(t, Gs, first=False):
                """LSTM pointwise chain from gate pre-activations ([g|i] and
                [f|o] tiles, i/f/o pre-scaled by 0.5); returns h2 (bf16)."""
                nt = int(n_t[t])
                r = slice(0, nt)
                tgi = work.tile([P, 2 * H], BF16, tag="tgi")
                nc.scalar.activation(tgi[r, :], Gs[0][r, :], TANH)
                si = work.tile([P, H], BF16, tag="si")
                nc.vector.tensor_scalar(si[r, :], tgi[r, H:2 * H], 1.0, 0.5, ADD, MULT)
                ig = work.tile([P, H], BF16, tag="ig")
                nc.vector.tensor_mul(ig[r, :], si[r, :], tgi[r, 0:H])
                tfo = work.tile([P, 2 * H], BF16, tag="tfo")
                nc.scalar.activation(tfo[r, :], Gs[1][r, :], TANH)
                if first:
                    nc.vector.tensor_copy(c_sb[r, :], ig[r, :])
                else:
                    sf = work.tile([P, H], BF16, tag="sf")
                    nc.vector.tensor_scalar(sf[r, :], tfo[r, 0:H], 1.0, 0.5, ADD, MULT)
                    fc = work.tile([P, H], BF16, tag="fc")
                    nc.vector.tensor_mul(fc[r, :], sf[r, :], c_sb[r, :])
                    nc.vector.tensor_add(c_sb[r, :], fc[r, :], ig[r, :])
                tc_ = work.tile([P, H], BF16, tag="tanhc")
                nc.scalar.activation(tc_[r, :], c_sb[r, :], TANH)
                so = work.tile([P, H], BF16, tag="so")
                nc.vector.tensor_scalar(so[r, :], tfo[r, H:2 * H], 1.0, 0.5, ADD, MULT)
                h2 = work.tile([P, H], BF16, tag="h2")
                nc.vector.tensor_mul(h2[r, :], so[r, :], tc_[r, :])
                return h2

            def pointwise_store(t, h2, stage, stage8, col0):
                """PE-transpose h2 into the stage tiles (emitted so the PE
                reaches it right as h2 lands)."""
                nt = int(n_t[t])
                pst = ps_tr.tile([P, 4 * P], BF16, tag="tr")
                for m in range(KH):
                    nc.tensor.transpose(pst[:, m * P:(m + 1) * P],
                                        h2[:, m * P:(m + 1) * P], ident16)
                pst3 = pst.rearrange("p (m b) -> p m b", m=KH)
                nc.vector.tensor_copy(stage8[:, :, col0:col0 + nt], pst3[:, :, 0:nt])
                nc.vector.tensor_copy(stage[:, :, col0:col0 + nt], pst3[:, :, 0:nt])

            # --- spread-out batched output projection ---------------------
            pending = []          # chunks not yet emitted: (rec, n0, n1)
            class _Flush:
                __slots__ = ("stage", "lg", "rows", "segments", "left")

            def queue_flush(stage, segments):
                rec = _Flush()
                rec.stage = stage
                rec.segments = segments
                rec.rows = segments[-1][2]
                rec.lg = work.tile([P, VS], BF16, tag="lg", bufs=3, name="lg")
                rec.left = 0
                for n0 in range(0, VS, 512):
                    pending.append((rec, n0, min(n0 + 512, VS)))
                    rec.left += 1

            def emit_chunk():
                """One 512-col output-projection chunk (fp8 DoubleRow); the
                PSUM->SBUF copy runs on the ACT engine (no table needed)."""
                rec, n0, n1 = pending.pop(0)
                rows = rec.rows
                ps = ps_o.tile([P, 512], F32, tag="o512")
                for k in range(KH):
                    nc.tensor.matmul(ps[0:rows, 0:n1 - n0],
                                     rec.stage[:, k, 0:rows], owt_sb[:, k, n0:n1],
                                     start=(k == 0), stop=(k == KH - 1))
                nc.scalar.activation(rec.lg[0:rows, n0:n1], ps[0:rows, 0:n1 - n0],
                                     COPY)
                rec.left -= 1
                if rec.left == 0:
                    for (ti_, r0, r1) in rec.segments:
                        nc.sync.dma_start(out[ti_, 0:r1 - r0, :], rec.lg[r0:r1, :])

            # ---------------- step 0 (gates precomputed on host) ----------------
            cur, col0 = 0, plan[0][0]
            g0_halves = [g0_sb[:, 0:1024], g0_sb[:, 1024:2048]]
            h2 = pointwise_compute(0, g0_halves, first=True)
            pointwise_store(0, h2, stages[cur], stages8[cur], col0)

            xT_next, x8_next = fetch_x(1)
            S_next = start_scores(1, x8_next)
            G_next = start_gates(1, x8_next)
            xT_fut = fetch_x(2)

            # ---------------- recurrence ----------------
            prev_stage, prev_stage8, prev_col = stages[cur], stages8[cur], col0
            for t in range(1, T):
                nt = int(n_t[t])
                col0, flush = plan[t]
                if flush is not None:
                    flush_stage = stages[cur]
                    cur ^= 1
                S, Gs = S_next, G_next
                xT_next, x8_next = xT_fut

                # finish attention scores: + att_Wh.T @ h_{t-1} (fp8 DR)
                for m in range(MA):
                    for j in range(KH // 2):
                        nc.tensor.matmul(S[:, m, 0:nt],
                                         awh_sb[:, 2 * j:2 * j + 2, m * P:(m + 1) * P],
                                         prev_stage8[:, 2 * j:2 * j + 2,
                                                     prev_col:prev_col + nt],
                                         start=False, stop=(j == KH // 2 - 1),
                                         perf_mode=DR)
                # softmax (deferred normalization)
                sc = work.tile([P, KA, B], BF16, tag="sc")
                nc.scalar.activation(sc[:, :, 0:nt], S[:, :, 0:nt], EXP)
                # projection chunks fill the PE while softmax runs; drain all
                # before this step's store rewrites the old stage on flushes
                if flush is not None:
                    while pending:
                        emit_chunk()
                elif pending:
                    emit_chunk()
                trt = ps_tr.tile([P, 512], F32, tag="tr")
                for m in range(MA):
                    nc.tensor.matmul(trt[0:1, 0:nt], ones_bf, sc[:, m, 0:nt],
                                     start=(m == 0), stop=(m == MA - 1))
                rden = work.tile([1, B], F32, tag="rden")
                nc.vector.reciprocal(rden[:, 0:nt], trt[0:1, 0:nt])
                rden_bf = work.tile([1, B], BF16, tag="rdenb")
                nc.vector.tensor_copy(rden_bf[:, 0:nt], rden[:, 0:nt])
                nc.tensor.matmul(trt[:, 128:128 + nt], ones_row.to_broadcast([1, P]),
                                 rden_bf[:, 0:nt], start=True, stop=True)
                attn = work.tile([P, KA, B], BF16, tag="attn")
                nc.vector.tensor_mul(attn[:, :, 0:nt], sc[:, :, 0:nt],
                                     cnn_sb[:, :, 0:nt])
                attn8 = work.tile([P, KA, B], F8, tag="attn8")
                nc.vector.tensor_tensor(
                    attn8[:, :, 0:nt], attn[:, :, 0:nt],
                    trt[:, 128:256].rearrange("p (k b) -> p k b", k=1)[:, :, 0:nt]
                    .to_broadcast([P, KA, nt]),
                    op=MULT)

                # gates, then the pointwise chain chunk-by-chunk
                gates_finish(t, Gs, attn8, prev_stage8, prev_col)
                h2 = pointwise_compute(t, Gs)

                if flush is not None:
                    queue_flush(flush_stage, flush)

                # next step's x contributions run in the pointwise PE window
                if t + 1 < T:
                    S_next = start_scores(t + 1, x8_next)
                    G_next = start_gates(t + 1, x8_next)
                # one more projection chunk plugs the px->h2 wait
                if pending:
                    emit_chunk()
                pointwise_store(t, h2, stages[cur], stages8[cur], col0)
                if t + 2 < T:
                    xT_fut = fetch_x(t + 2)

                prev_stage, prev_stage8, prev_col = stages[cur], stages8[cur], col0

            queue_flush(stages[cur], final_segs)
            while pending:
                emit_chunk()

    nc.finalize()
    return nc


def _reorder_gates(w, axis):
    """Reorder the 4H gate dim from [i|f|g|o] (torch order) to [g|i|f|o]."""
    idx = np.concatenate([np.arange(2 * H, 3 * H), np.arange(0, H),
                          np.arange(H, 2 * H), np.arange(3 * H, 4 * H)])
    return np.take(w, idx, axis=axis)


def _prep_inputs(inputs):
    f = {k: np.asarray(v) for k, v in inputs.items()}
    lengths = f["lengths"].astype(np.int64)
    n_t = [int((lengths > t).sum()) for t in range(T)]

    att_W = np.asarray(f["att_W"], np.float32)
    attd_W = np.asarray(f["attd_W"], np.float32)
    W_ih = _reorder_gates(np.asarray(f["W_ih"], np.float32), axis=0)
    W_hh = _reorder_gates(np.asarray(f["W_hh"], np.float32), axis=0)
    b0 = _reorder_gates(np.asarray(f["b_ih"], np.float32)
                        + np.asarray(f["b_hh"], np.float32), axis=0)
    out_W = np.asarray(f["out_W"], np.float32)

    def bf(x):
        return np.ascontiguousarray(x.astype(NP_BF16))

    def f8(x):
        return np.ascontiguousarray(x.astype(NP_F8))

    # host-side fold matrices (fp32) for the fp8 gate GEMMs
    cx = attd_W[:, :E].T @ W_ih.T                     # (E, 4H)
    ca = attd_W[:, E:].T @ W_ih.T                     # (A, 4H)
    bc = np.asarray(f["attd_b"], np.float32) @ W_ih.T + b0   # (4H,)
    g0 = np.asarray(f["features"], np.float32) @ W_ih.T + b0  # (B, 4H)

    # fold the sigmoid half-angle scaling into the i/f/o gate columns
    # (gate order [g|i|f|o]: columns H:4H get 0.5)
    gs = np.ones((G4,), np.float32)
    gs[H:] = 0.5
    cx *= gs
    ca *= gs
    whh_s = W_hh.T * gs
    bc = bc * gs
    g0 = g0 * gs

    base = {
        "cnn_T": bf(np.asarray(f["cnn_features"], np.float32).T),
        "emb_W": bf(np.asarray(f["emb_W"], np.float32)),
        "awh": f8(att_W[:, E:].T),
        "awx": f8(att_W[:, :E].T),
        "attb_row": bf(np.asarray(f["att_b"], np.float32).reshape(1, A)),
        "cx8": f8(cx),
        "ca8": f8(ca),
        "whh8": f8(whh_s),
        "bc_row": bf(bc.reshape(1, G4)),
        "g0": np.ascontiguousarray(g0.astype(np.float32)),
    }

    caps = np.asarray(f["captions"], np.int64)          # (B, T-1)
    caps_pad = np.zeros((T, B), np.int32)
    caps_pad[:T - 1] = caps.T.astype(np.int32)          # caps_pad[t-1] = x_t tokens
    base["caps"] = np.ascontiguousarray(caps_pad)

    in_maps = []
    for c in range(NCORES):
        m = dict(base)
        m["owt"] = bf(out_W[c * VS:(c + 1) * VS].T)
        in_maps.append(m)
    return in_maps, n_t


_CACHE = {}


def kernel(**inputs):
    in_maps, n_t = _prep_inputs(inputs)
    key = tuple(n_t)
    if key not in _CACHE:
        _CACHE[key] = _build_nc(n_t)
    nc = _CACHE[key]
    res = run_bass_kernel_spmd(nc, in_maps, list(range(NCORES)))
    outs = [np.asarray(res.results[c]["out"]) for c in range(NCORES)]
    full = np.concatenate(outs, axis=-1).astype(np.float32)   # (T, B, V)
    full += np.asarray(inputs["out_b"], np.float32)[None, None, :]
    # device only writes the first n_t[t] (valid) rows of each step
    mask = np.arange(B)[None, :] < np.asarray(n_t)[:, None]   # (T, B)
    full[~mask] = 0.0
    return full


# revision 18
# speedup vs baseline: 1.1728x; 1.1728x over previous
"""Trainium2 Bass kernel for nn_DecoderRNN (attention LSTM decoder + vocab projection).

Strategy (8 NeuronCores):
  - The 63-step LSTM/attention recurrence is replicated on all cores (identical
    SPMD program); the dominant output projection (T*B, H) x (H, V) is sharded
    over the vocab dimension (V/8 = 1250 logit columns per core). No collectives.
  - fp8-e4m3 + DoubleRow perf mode (2 contraction rows per partition, halving
    the instruction stream) for every recurrence GEMM: gates (x@Cx,
    attended@Ca, h@W_hh.T), attention scores, and the output projection.
    Fold matrices Cx/Ca and the step-0 gates are precomputed on the host in
    fp32.
  - Gate columns are ordered [g|i|f|o] and each 512-wide gate lives in its OWN
    single-bank PSUM tile, so tanh(g)/sigmoid(i) start as soon as their chunk
    of the gate GEMM finishes instead of after the full stream.
  - Per-step x-contributions (PA, PX) and all biases are accumulated directly
    into those PSUM banks one step ahead (start/stop accumulation groups).
  - sigmoid(x) = 0.5*tanh(x/2)+0.5 keeps every activation on the exp/tanh
    table: zero ACT table reloads in the loop.
  - h is packed column-wise (feature-major) into staging tiles; the output
    projection runs on 128-row batches, spread across steps' PE idle windows;
    its PSUM->SBUF copies run on the ACT engine (Copy needs no table).
  - Logits are written bf16, valid rows only; the host zero-fills, upcasts,
    and adds the output bias.  Ragged lengths are baked into the instruction
    stream.
"""

import os
import sys

import numpy as np

for _p in ("/opt/trn_rl_repo", "/root/.axon_site/_ro/trn_rl_repo"):
    if os.path.isdir(_p) and _p not in sys.path:
        sys.path.insert(0, _p)

import ml_dtypes
import concourse.bass as bass
import concourse.tile as tile
from concourse import bacc, mybir
from concourse.bass_utils import run_bass_kernel_spmd
from concourse.masks import make_identity

F32 = mybir.dt.float32
BF16 = mybir.dt.bfloat16
F8 = mybir.dt.float8e4
I32 = mybir.dt.int32
ADD = mybir.AluOpType.add
MULT = mybir.AluOpType.mult
TANH = mybir.ActivationFunctionType.Tanh
EXP = mybir.ActivationFunctionType.Exp
COPY = mybir.ActivationFunctionType.Copy
DR = mybir.MatmulPerfMode.DoubleRow
NP_BF16 = ml_dtypes.bfloat16
NP_F8 = np.dtype(mybir.dt.np(F8))

B, T, E, H, A, V = 128, 64, 512, 512, 512, 10000
G4 = 4 * H                      # 2048
NCORES = 8
VS = V // NCORES                # 1250 vocab columns per core
P = 128

KE = E // P                     # 4 k-tiles over E
KH = H // P
KA = A // P
MA = A // P                     # A m-tiles (feature-major attention)
NCH = 4                         # four 512-wide gate chunks: [g|i|f|o]


def _flush_plan(n_t):
    """Pack per-step h rows into 128-row batches for the output projection."""
    plan = []          # per t: (col0, flush_before: segments or None)
    segs = []
    pos = 0
    for t in range(T):
        nt = int(n_t[t])
        flush = None
        if pos + nt > P:
            flush = segs
            segs = []
            pos = 0
        plan.append((pos, flush))
        segs.append((t, pos, pos + nt))
        pos += nt
    return plan, segs  # segs = final leftover batch


def _build_nc(n_t):
    nc = bacc.Bacc("TRN2", target_bir_lowering=False, debug=False,
                   num_devices=NCORES)

    # ---------------- I/O ----------------
    cnn_T = nc.declare_dram_parameter("cnn_T", [A, B], BF16, isOutput=False)
    caps = nc.declare_dram_parameter("caps", [T, B], I32, isOutput=False)
    emb_W = nc.declare_dram_parameter("emb_W", [V, E], BF16, isOutput=False)
    awh_d = nc.declare_dram_parameter("awh", [H, A], BF16, isOutput=False)
    awx_d = nc.declare_dram_parameter("awx", [E, A], BF16, isOutput=False)
    attb_row = nc.declare_dram_parameter("attb_row", [1, A], BF16, isOutput=False)
    cx8_d = nc.declare_dram_parameter("cx8", [E, G4], F8, isOutput=False)
    ca8_d = nc.declare_dram_parameter("ca8", [A, G4], F8, isOutput=False)
    whh8_d = nc.declare_dram_parameter("whh8", [H, G4], F8, isOutput=False)
    bc_row = nc.declare_dram_parameter("bc_row", [1, G4], BF16, isOutput=False)
    g0_d = nc.declare_dram_parameter("g0", [B, G4], F32, isOutput=False)
    owt_d = nc.declare_dram_parameter("owt", [H, VS], BF16, isOutput=False)
    out = nc.declare_dram_parameter("out", [T, B, VS], BF16, isOutput=True)

    plan, final_segs = _flush_plan(n_t)

    with tile.TileContext(nc) as tc:
        with (
            tc.tile_pool(name="consts", bufs=1) as consts,
            tc.tile_pool(name="state", bufs=1) as state,
            tc.tile_pool(name="work", bufs=2) as work,
            tc.tile_pool(name="xstream", bufs=3) as xstream,
            tc.tile_pool(name="ps_g", bufs=1, space="PSUM") as ps_g,    # 4 banks
            tc.tile_pool(name="ps_s", bufs=2, space="PSUM") as ps_s,    # 2 banks
            tc.tile_pool(name="ps_tr", bufs=1, space="PSUM") as ps_tr,  # 1 bank
            tc.tile_pool(name="ps_o", bufs=1, space="PSUM") as ps_o,    # 1 bank
        ):
            # ---------------- weight / const loads (two HWDGE queues) ----------------
            ident16 = consts.tile([P, P], BF16)
            make_identity(nc, ident16)
            ones_bf = consts.tile([P, 1], BF16)
            nc.vector.memset(ones_bf, 1.0)

            def load3(dst, dram_ap):
                nc.sync.dma_start(dst, dram_ap.rearrange("(k p) n -> p k n", p=P))

            def load3b(dst, dram_ap):
                nc.scalar.dma_start(dst, dram_ap.rearrange("(k p) n -> p k n", p=P))

            g0_sb = consts.tile([P, G4], F32)
            nc.sync.dma_start(g0_sb, g0_d[:, :])
            toks = state.tile([B, T], I32)
            nc.sync.dma_start(toks, caps[:, :].rearrange("t b -> b t"))
            cnn_sb = consts.tile([P, KA, B], BF16)
            load3(cnn_sb, cnn_T[:, :])
            attb_sb = consts.tile([1, A], BF16)
            nc.sync.dma_start(attb_sb, attb_row[:, :])
            bc_sb = consts.tile([1, G4], BF16)
            nc.sync.dma_start(bc_sb, bc_row[:, :])

            awh_sb = state.tile([P, KH, A], BF16)
            load3(awh_sb, awh_d[:, :])
            awx_sb = state.tile([P, KE, A], BF16)
            load3(awx_sb, awx_d[:, :])
            cx8_sb = state.tile([P, KE, G4], F8)
            load3(cx8_sb, cx8_d[:, :])
            ca8_sb = state.tile([P, KA, G4], F8)
            load3b(ca8_sb, ca8_d[:, :])
            whh8_sb = state.tile([P, KH, G4], F8)
            load3b(whh8_sb, whh8_d[:, :])
            owt_sb = state.tile([P, KH, VS], BF16)
            load3b(owt_sb, owt_d[:, :])

            # recurrent state
            c_sb = state.tile([P, H], BF16)           # c, B-major
            stages = [state.tile([P, KH, P], BF16, name=f"stage{i}")
                      for i in range(2)]
            stages8 = [state.tile([P, KH, P], F8, name=f"stage8_{i}")
                       for i in range(2)]

            ones_row = ones_bf[0:1, 0:1]

            # ---------------- helpers ----------------
            def fetch_x(t):
                """Gather x_t embeddings; bf16 [E(part), KE, B] + fp8 cast."""
                xg = xstream.tile([P, E], BF16, tag="xg")
                nc.gpsimd.indirect_dma_start(
                    out=xg, out_offset=None, in_=emb_W[:, :],
                    in_offset=bass.IndirectOffsetOnAxis(ap=toks[:, t - 1:t], axis=0))
                xT = xstream.tile([P, KE, B], BF16, tag="xT")
                nc.sync.dma_start_transpose(xT, xg)
                x8 = xstream.tile([P, KE, B], F8, tag="x8")
                nc.vector.tensor_copy(x8, xT)
                return xT, x8

            def start_scores(t, xT):
                """New PSUM score tile for step t: att_b + PA (bf16)."""
                nt = int(n_t[t])
                S = ps_s.tile([P, MA, B], F32, tag="att")
                for m in range(MA):
                    nc.tensor.matmul(S[:, m, 0:nt],
                                     attb_sb[0:1, m * P:(m + 1) * P],
                                     ones_row.to_broadcast([1, nt]),
                                     start=True, stop=False)
                    for k in range(KE):
                        nc.tensor.matmul(S[:, m, 0:nt],
                                         awx_sb[:, k, m * P:(m + 1) * P],
                                         xT[:, k, 0:nt], start=False, stop=False)
                return S

            def start_gates(t, x8):
                """Two new 2-bank PSUM gate tiles ([g|i] and [f|o]) for step t,
                seeded with bc + PX (fp8 DoubleRow)."""
                nt = int(n_t[t])
                Gs = []
                for half in range(2):
                    Gh = ps_g.tile([P, 1024], F32, tag=f"g{half}", name=f"g{half}")
                    for ci in range(2):
                        ns = slice((2 * half + ci) * 512, (2 * half + ci + 1) * 512)
                        rg = slice(ci * 512, (ci + 1) * 512)
                        nc.tensor.matmul(Gh[0:nt, rg], ones_row.to_broadcast([1, nt]),
                                         bc_sb[0:1, ns], start=True, stop=False)
                        for j in range(KE // 2):
                            nc.tensor.matmul(Gh[0:nt, rg], x8[:, 2 * j:2 * j + 2, 0:nt],
                                             cx8_sb[:, 2 * j:2 * j + 2, ns],
                                             start=False, stop=False, perf_mode=DR)
                    Gs.append(Gh)
                return Gs

            def gates_finish(t, Gs, attn8, hstage8, hcol):
                """+= attended @ Ca + h @ W_hh.T, one gate chunk at a time so
                downstream ACTs start as early as possible."""
                nt = int(n_t[t])
                for ci in range(NCH):
                    ns = slice(ci * 512, (ci + 1) * 512)
                    Gc = Gs[ci // 2]
                    rg = slice((ci % 2) * 512, (ci % 2 + 1) * 512)
                    for j in range(KA // 2):
                        nc.tensor.matmul(Gc[0:nt, rg], attn8[:, 2 * j:2 * j + 2, 0:nt],
                                         ca8_sb[:, 2 * j:2 * j + 2, ns],
                                         start=False, stop=False, perf_mode=DR)
                    for j in range(KH // 2):
                        nc.tensor.matmul(Gc[0:nt, rg],
                                         hstage8[:, 2 * j:2 * j + 2, hcol:hcol + nt],
                                         whh8_sb[:, 2 * j:2 * j + 2, ns],
                                         start=False, stop=(j == KH // 2 - 1),
                                         perf_mode=DR)

            def pointwise_compute(t, Gs, first=False):
                """LSTM pointwise chain from gate pre-activations ([g|i] and
                [f|o] tiles, i/f/o pre-scaled by 0.5); returns h2 (bf16)."""
                nt = int(n_t[t])
                r = slice(0, nt)
                tgi = work.tile([P, 2 * H], BF16, tag="tgi")
                nc.scalar.activation(tgi[r, :], Gs[0][r, :], TANH)
                si = work.tile([P, H], BF16, tag="si")
                nc.vector.tensor_scalar(si[r, :], tgi[r, H:2 * H], 1.0, 0.5, ADD, MULT)
                ig = work.tile([P, H], BF16, tag="ig")
                nc.vector.tensor_mul(ig[r, :], si[r, :], tgi[r, 0:H])
                tfo = work.tile([P, 2 * H], BF16, tag="tfo")
                nc.scalar.activation(tfo[r, :], Gs[1][r, :], TANH)
                if first:
                    nc.vector.tensor_copy(c_sb[r, :], ig[r, :])
                else:
                    sf = work.tile([P, H], BF16, tag="sf")
                    nc.vector.tensor_scalar(sf[r, :], tfo[r, 0:H], 1.0, 0.5, ADD, MULT)
                    fc = work.tile([P, H], BF16, tag="fc")
                    nc.vector.tensor_mul(fc[r, :], sf[r, :], c_sb[r, :])
                    nc.vector.tensor_add(c_sb[r, :], fc[r, :], ig[r, :])
                tc_ = work.tile([P, H], BF16, tag="tanhc")
                nc.scalar.activation(tc_[r, :], c_sb[r, :], TANH)
                so = work.tile([P, H], BF16, tag="so")
                nc.vector.tensor_scalar(so[r, :], tfo[r, H:2 * H], 1.0, 0.5, ADD, MULT)
                h2 = work.tile([P, H], BF16, tag="h2")
                nc.vector.tensor_mul(h2[r, :], so[r, :], tc_[r, :])
                return h2

            def pointwise_store(t, h2, stage, stage8, col0):
                """PE-transpose h2 into the stage tiles (emitted so the PE
                reaches it right as h2 lands)."""
                nt = int(n_t[t])
                pst = ps_tr.tile([P, 4 * P], BF16, tag="tr")
                for m in range(KH):
                    nc.tensor.transpose(pst[:, m * P:(m + 1) * P],
                                        h2[:, m * P:(m + 1) * P], ident16)
                pst3 = pst.rearrange("p (m b) -> p m b", m=KH)
                nc.vector.tensor_copy(stage8[:, :, col0:col0 + nt], pst3[:, :, 0:nt])
                nc.vector.tensor_copy(stage[:, :, col0:col0 + nt], pst3[:, :, 0:nt])

            # --- spread-out batched output projection ---------------------
            pending = []          # chunks not yet emitted: (rec, n0, n1)
            class _Flush:
                __slots__ = ("stage", "lg", "rows", "segments", "left")

            def queue_flush(stage, segments):
                rec = _Flush()
                rec.stage = stage
                rec.segments = segments
                rec.rows = segments[-1][2]
                rec.lg = work.tile([P, VS], BF16, tag="lg", bufs=3, name="lg")
                rec.left = 0
                for n0 in range(0, VS, 512):
                    pending.append((rec, n0, min(n0 + 512, VS)))
                    rec.left += 1

            def emit_chunk():
                """One 512-col output-projection chunk (fp8 DoubleRow); the
                PSUM->SBUF copy runs on the ACT engine (no table needed)."""
                rec, n0, n1 = pending.pop(0)
                rows = rec.rows
                ps = ps_o.tile([P, 512], F32, tag="o512")
                for k in range(KH):
                    nc.tensor.matmul(ps[0:rows, 0:n1 - n0],
                                     rec.stage[:, k, 0:rows], owt_sb[:, k, n0:n1],
                                     start=(k == 0), stop=(k == KH - 1))
                nc.scalar.activation(rec.lg[0:rows, n0:n1], ps[0:rows, 0:n1 - n0],
                                     COPY)
                rec.left -= 1
                if rec.left == 0:
                    for (ti_, r0, r1) in rec.segments:
                        nc.sync.dma_start(out[ti_, 0:r1 - r0, :], rec.lg[r0:r1, :])

            # ---------------- step 0 (gates precomputed on host) ----------------
            cur, col0 = 0, plan[0][0]
            g0_halves = [g0_sb[:, 0:1024], g0_sb[:, 1024:2048]]
            h2 = pointwise_compute(0, g0_halves, first=True)
            pointwise_store(0, h2, stages[cur], stages8[cur], col0)

            xT_next, x8_next = fetch_x(1)
            S_next = start_scores(1, xT_next)
            G_next = start_gates(1, x8_next)
            xT_fut = fetch_x(2)

            # ---------------- recurrence ----------------
            prev_stage, prev_stage8, prev_col = stages[cur], stages8[cur], col0
            for t in range(1, T):
                nt = int(n_t[t])
                col0, flush = plan[t]
                if flush is not None:
                    flush_stage = stages[cur]
                    cur ^= 1
                S, Gs = S_next, G_next
                xT_next, x8_next = xT_fut

                # finish attention scores: + att_Wh.T @ h_{t-1}
                for m in range(MA):
                    for k in range(KH):
                        nc.tensor.matmul(S[:, m, 0:nt],
                                         awh_sb[:, k, m * P:(m + 1) * P],
                                         prev_stage[:, k, prev_col:prev_col + nt],
                                         start=False, stop=(k == KH - 1))
                # softmax (deferred normalization)
                sc = work.tile([P, KA, B], BF16, tag="sc")
                nc.scalar.activation(sc[:, :, 0:nt], S[:, :, 0:nt], EXP)
                # projection chunks fill the PE while softmax runs; drain all
                # before this step's store rewrites the old stage on flushes
                if flush is not None:
                    while pending:
                        emit_chunk()
                elif pending:
                    emit_chunk()
                trt = ps_tr.tile([P, 512], F32, tag="tr")
                for m in range(MA):
                    nc.tensor.matmul(trt[0:1, 0:nt], ones_bf, sc[:, m, 0:nt],
                                     start=(m == 0), stop=(m == MA - 1))
                rden = work.tile([1, B], F32, tag="rden")
                nc.vector.reciprocal(rden[:, 0:nt], trt[0:1, 0:nt])
                rden_bf = work.tile([1, B], BF16, tag="rdenb")
                nc.vector.tensor_copy(rden_bf[:, 0:nt], rden[:, 0:nt])
                nc.tensor.matmul(trt[:, 128:128 + nt], ones_row.to_broadcast([1, P]),
                                 rden_bf[:, 0:nt], start=True, stop=True)
                attn = work.tile([P, KA, B], BF16, tag="attn")
                nc.vector.tensor_mul(attn[:, :, 0:nt], sc[:, :, 0:nt],
                                     cnn_sb[:, :, 0:nt])
                attn8 = work.tile([P, KA, B], F8, tag="attn8")
                nc.vector.tensor_tensor(
                    attn8[:, :, 0:nt], attn[:, :, 0:nt],
                    trt[:, 128:256].rearrange("p (k b) -> p k b", k=1)[:, :, 0:nt]
                    .to_broadcast([P, KA, nt]),
                    op=MULT)

                # gates, then the pointwise chain chunk-by-chunk
                gates_finish(t, Gs, attn8, prev_stage8, prev_col)
                h2 = pointwise_compute(t, Gs)

                if flush is not None:
                    queue_flush(flush_stage, flush)

                # next step's x contributions run in the pointwise PE window
                if t + 1 < T:
                    S_next = start_scores(t + 1, xT_next)
                    G_next = start_gates(t + 1, x8_next)
                # one more projection chunk plugs the px->h2 wait
                if pending:
                    emit_chunk()
                pointwise_store(t, h2, stages[cur], stages8[cur], col0)
                if t + 2 < T:
                    xT_fut = fetch_x(t + 2)

                prev_stage, prev_stage8, prev_col = stages[cur], stages8[cur], col0

            queue_flush(stages[cur], final_segs)
            while pending:
                emit_chunk()

    nc.finalize()
    return nc


def _reorder_gates(w, axis):
    """Reorder the 4H gate dim from [i|f|g|o] (torch order) to [g|i|f|o]."""
    idx = np.concatenate([np.arange(2 * H, 3 * H), np.arange(0, H),
                          np.arange(H, 2 * H), np.arange(3 * H, 4 * H)])
    return np.take(w, idx, axis=axis)


def _prep_inputs(inputs):
    f = {k: np.asarray(v) for k, v in inputs.items()}
    lengths = f["lengths"].astype(np.int64)
    n_t = [int((lengths > t).sum()) for t in range(T)]

    att_W = np.asarray(f["att_W"], np.float32)
    attd_W = np.asarray(f["attd_W"], np.float32)
    W_ih = _reorder_gates(np.asarray(f["W_ih"], np.float32), axis=0)
    W_hh = _reorder_gates(np.asarray(f["W_hh"], np.float32), axis=0)
    b0 = _reorder_gates(np.asarray(f["b_ih"], np.float32)
                        + np.asarray(f["b_hh"], np.float32), axis=0)
    out_W = np.asarray(f["out_W"], np.float32)

    def bf(x):
        return np.ascontiguousarray(x.astype(NP_BF16))

    def f8(x):
        return np.ascontiguousarray(x.astype(NP_F8))

    # host-side fold matrices (fp32) for the fp8 gate GEMMs
    cx = attd_W[:, :E].T @ W_ih.T                     # (E, 4H)
    ca = attd_W[:, E:].T @ W_ih.T                     # (A, 4H)
    bc = np.asarray(f["attd_b"], np.float32) @ W_ih.T + b0   # (4H,)
    g0 = np.asarray(f["features"], np.float32) @ W_ih.T + b0  # (B, 4H)

    # fold the sigmoid half-angle scaling into the i/f/o gate columns
    # (gate order [g|i|f|o]: columns H:4H get 0.5)
    gs = np.ones((G4,), np.float32)
    gs[H:] = 0.5
    cx *= gs
    ca *= gs
    whh_s = W_hh.T * gs
    bc = bc * gs
    g0 = g0 * gs

    base = {
        "cnn_T": bf(np.asarray(f["cnn_features"], np.float32).T),
        "emb_W": bf(np.asarray(f["emb_W"], np.float32)),
        "awh": bf(att_W[:, E:].T),
        "awx": bf(att_W[:, :E].T),
        "attb_row": bf(np.asarray(f["att_b"], np.float32).reshape(1, A)),
        "cx8": f8(cx),
        "ca8": f8(ca),
        "whh8": f8(whh_s),
        "bc_row": bf(bc.reshape(1, G4)),
        "g0": np.ascontiguousarray(g0.astype(np.float32)),
    }

    caps = np.asarray(f["captions"], np.int64)          # (B, T-1)
    caps_pad = np.zeros((T, B), np.int32)
    caps_pad[:T - 1] = caps.T.astype(np.int32)          # caps_pad[t-1] = x_t tokens
    base["caps"] = np.ascontiguousarray(caps_pad)

    in_maps = []
    for c in range(NCORES):
        m = dict(base)
        m["owt"] = bf(out_W[c * VS:(c + 1) * VS].T)
        in_maps.append(m)
    return in_maps, n_t


_CACHE = {}


def kernel(**inputs):
    in_maps, n_t = _prep_inputs(inputs)
    key = tuple(n_t)
    if key not in _CACHE:
        _CACHE[key] = _build_nc(n_t)
    nc = _CACHE[key]
    res = run_bass_kernel_spmd(nc, in_maps, list(range(NCORES)))
    outs = [np.asarray(res.results[c]["out"]) for c in range(NCORES)]
    full = np.concatenate(outs, axis=-1).astype(np.float32)   # (T, B, V)
    full += np.asarray(inputs["out_b"], np.float32)[None, None, :]
    # device only writes the first n_t[t] (valid) rows of each step
    mask = np.arange(B)[None, :] < np.asarray(n_t)[:, None]   # (T, B)
    full[~mask] = 0.0
    return full
